# revision 36
# baseline (speedup 1.0000x reference)
"""Trainium2 Bass kernel: GQA multi-head self-attention (B=1, L=4096, D=1024,
16 Q heads, 4 KV heads, head_dim 64, interleaved RoPE, causal softmax).

Sharding: 2 query heads + their (shared) KV head per core, 8 cores.
Each core computes a full-shape partial output Y_c.T = (attn_c @ Wo_c.T).T
(Megatron row-parallel style); the host sums the 8 partials.

Device-side design (per core):
  - x is fed pre-transposed (xT [D, L], fp16) so projection matmuls stream
    natural SBUF tiles; matmul operands are fp16 (1 cycle/row on the PE),
    accumulation stays fp32 in PSUM.
  - Q.T/K.T are produced in a "half-split" head-dim order (even dims then odd
    dims per head, via host-permuted weight rows) so RoPE's rotate-pair becomes
    a 32-partition block swap, done with SBUF->SBUF DMAs.
  - Attention runs in the S.T = K @ Q.T orientation: scores land in PSUM as
    [k=128, 2, q] tiles (both heads in one tile), exp runs on the scalar
    engine straight out of PSUM, and PV uses [V | ones] as the stationary
    operand so softmax denominators come out as row 64 of the PV accumulator
    for free. Diagonal key-blocks compute only the causally live query columns
    (matmul, exp and PV all narrowed).
  - Softmax normalization: DVE reciprocal straight off the PSUM denominator
    row, gpsimd partition_broadcast to replicate it across 64 partitions, one
    fused [128, q] attention-out tile so the output projection is 8 single
    (contraction-128) matmuls per chunk.
  - No max-subtraction pass: scores are O(1) here, exp cannot overflow, and
    softmax is shift-invariant so the result matches the reference.
  - Chunks have variable width: the first 512 columns are processed as two
    256-wide chunks so the startup pipeline (x DMA -> proj -> RoPE -> QK)
    fills in half the time; early-chunk PSUM evacuation runs on the (then
    idle) scalar engine to shorten the serial DVE RoPE chain.
  - Emission is software-pipelined: QK^T/exp run two key-blocks ahead of PV,
    and each chunk's normalize + output projection is deferred until the next
    chunk's first key-blocks are in flight, its 8 output-projection matmuls
    spread one-per-key-block so the PSUM ring never blocks the in-order PE
    stream. Non-final chunks store the projected output in one [128, 8, qw]
    staging tile and issue a single batched DMA; the final chunk uses paired
    PSUM tiles with alternating DVE/scalar evacuation and SP/gpsimd DMA
    queues to shorten the drain tail.
"""

import sys

for _p in ("/opt/trn_rl_repo",):
    if _p not in sys.path:
        sys.path.insert(0, _p)

import numpy as np

import concourse.bacc as bacc
import concourse.mybir as mybir
import concourse.tile as tile
from concourse.bass_utils import run_bass_kernel_spmd

F32 = mybir.dt.float32
F16 = mybir.dt.float16

D_MODEL = 1024
NUM_HEADS = 16
NUM_KV_HEADS = 4
HEAD_DIM = 64
THETA = 10000.0
N_CORES = 8
QC = 512          # max query chunk (free dim of S.T tiles per head)
KB = 128          # key block (partition dim of S.T tiles)


def chunk_bounds(L):
    """(q0, qw) per chunk: first 512 cols as two 256-wide chunks, then 512s."""
    out = [(0, 512)]
    q = 512
    while q < L:
        out.append((q, 512))
        q += 512
    return out


def build_kernel(L=4096):
    """One-core SPMD program. Handles its 2 query heads + 1 shared KV head."""
    nc = bacc.Bacc(None, target_bir_lowering=False)
    NT = L // KB          # number of 128-row key blocks / V tiles
    BOUNDS = chunk_bounds(L)
    NCH = len(BOUNDS)

    xt = nc.dram_tensor("xt", [D_MODEL, L], F16, kind="ExternalInput")
    wqt = nc.dram_tensor("wqt", [128, 8, 128], F16, kind="ExternalInput")
    wkvt = nc.dram_tensor("wkvt", [128, 8, 128], F16, kind="ExternalInput")
    wo01 = nc.dram_tensor("wo01", [128, 8, 128], F16, kind="ExternalInput")
    cs2 = nc.dram_tensor("cs2", [128, 2, L], F16, kind="ExternalInput")
    # aux packs [tri | identlo | cs0-swapped-S] so startup needs one small DMA
    aux = nc.dram_tensor("aux", [128, 192 + QC], F16, kind="ExternalInput")
    yt = nc.dram_tensor("yt", [D_MODEL, L], F16, kind="ExternalOutput")

    xt_r = xt.rearrange("(dc p) l -> p dc l", p=128)      # [128, 8, L]
    yt_r = yt.rearrange("(dc p) l -> p dc l", p=128)      # [128, 8, L]

    with tile.TileContext(nc) as tc:
        with (
            tc.tile_pool(name="consts", bufs=1) as consts,
            tc.tile_pool(name="big", bufs=1) as big,
            tc.tile_pool(name="xin", bufs=4) as xin,
            tc.tile_pool(name="work", bufs=5) as work,
            tc.tile_pool(name="ybp", bufs=2) as ybp,
            tc.tile_pool(name="ylast", bufs=8) as ylast,
            tc.tile_pool(name="ptp", bufs=14) as ptp,
            tc.tile_pool(name="stp", bufs=2, space="PSUM") as stp,
            tc.tile_pool(name="otp", bufs=2, space="PSUM") as otp,
            tc.tile_pool(name="mp", bufs=2, space="PSUM") as mp,
        ):
            # ---- constants in SBUF ----
            wqt_s = consts.tile([128, 8, 128], F16, tag="wqt")
            wkvt_s = consts.tile([128, 8, 128], F16, tag="wkvt")
            wo01_s = consts.tile([128, 8, 128], F16, tag="wo01")
            cs_s = consts.tile([128, 2, L], F16, tag="cs")
            aux_s = consts.tile([128, 192 + QC], F16, tag="aux")
            tri_s = aux_s[:, 0:128]
            identlo_s = aux_s[:, 128:192]
            cssw_s = aux_s[:, 192:192 + QC]

            def load_late_consts():
                nc.sync.dma_start(out=wo01_s, in_=wo01[:, :, :])

            # ---- persistent per-core activations ----
            qtrope = big.tile([128, L], F16, tag="qtrope")      # [2*64 halfsplit d, L]
            kt2 = big.tile([128, L], F16, tag="kt2")            # K.T duplicated twice
            vn = big.tile([128, NT * 65], F16, tag="vn")        # [V | 1] blocks
            nc.gpsimd.memset(vn[:, 64::65], 1.0)                # just the ones columns

            xtiles = {}

            def proj_dma(ci):
                q0, qw = BOUNDS[ci]
                ls = slice(q0, q0 + qw)
                xbig = xin.tile([128, 8, QC], F16, tag="xt")
                if ci == 0:
                    # startup ordering: DMAs staged in matmul-consumption order
                    # (q dc0-3, kv dc0-3, q dc4-7, kv dc4-7) at fine grain so
                    # the first projection matmul starts after ~0.3MB
                    nc.sync.dma_start(out=wqt_s[:, 0:4, :], in_=wqt[:, 0:4, :])
                    nc.sync.dma_start(out=xbig[:, 0:4, 0:qw], in_=xt_r[:, 0:4, ls])
                    nc.sync.dma_start(out=wkvt_s[:, 0:4, :], in_=wkvt[:, 0:4, :])
                    nc.sync.dma_start(out=wqt_s[:, 4:8, :], in_=wqt[:, 4:8, :])
                    nc.sync.dma_start(out=xbig[:, 4:8, 0:qw], in_=xt_r[:, 4:8, ls])
                    nc.sync.dma_start(out=wkvt_s[:, 4:8, :], in_=wkvt[:, 4:8, :])
                    nc.sync.dma_start(out=cs_s[:, 0, ls], in_=cs2[:, 0, ls])
                    nc.sync.dma_start(out=aux_s, in_=aux[:, :])
                else:
                    nc.sync.dma_start(out=xbig[:, :, 0:qw], in_=xt_r[:, :, ls])
                    nc.sync.dma_start(out=cs_s[:, :, ls], in_=cs2[:, :, ls])
                xtiles[ci] = xbig

            def make_proj_pieces(ci):
                q0, qw = BOUNDS[ci]
                ls = slice(q0, q0 + qw)
                xbig = xtiles.pop(ci)
                psum = {}

                def mm_piece(kind, dc):
                    def run():
                        if not psum:
                            psum["qt"] = mp.tile([128, QC], F32, tag="mp",
                                                  name="qt_ps")
                            psum["kv"] = mp.tile([128, QC], F32, tag="mp",
                                                  name="kvt_ps")
                        ps = psum["qt"] if kind == "q" else psum["kv"]
                        w = wqt_s if kind == "q" else wkvt_s
                        nc.tensor.matmul(ps[:, 0:qw], w[:, dc, :],
                                         xbig[:, dc, 0:qw],
                                         start=(dc == 0), stop=(dc == 7))
                    return run

                # half-interleaved so the low x half can be consumed while the
                # high half's DMA is still in flight (matters for chunk 0)
                pieces = ([mm_piece("q", dc) for dc in range(4)]
                          + [mm_piece("kv", dc) for dc in range(4)]
                          + [mm_piece("q", dc) for dc in range(4, 8)]
                          + [mm_piece("kv", dc) for dc in range(4, 8)])

                early = q0 + qw <= QC
                fill_phase = q0 + qw <= 3 * QC
                sbuf = {}

                def tail_dve():
                    # evacuate PSUM (fp32 -> fp16); early chunks route the
                    # Q-side evac to the (idle) scalar engine so the DVE RoPE
                    # chain starts sooner
                    qtraw = work.tile([128, QC], F16, tag="qtraw")
                    kvts = work.tile([128, QC], F16, tag="kvts")
                    sbuf["kvts"] = kvts
                    nc.vector.tensor_copy(kvts[:, 0:qw], psum["kv"][:, 0:qw])
                    if fill_phase:
                        nc.scalar.copy(qtraw[:, 0:qw], psum["qt"][:, 0:qw])
                    else:
                        nc.vector.tensor_copy(qtraw[:, 0:qw], psum["qt"][:, 0:qw])
                    rope(qtraw, kvts)

                def vt_piece(t):
                    # V natural layout via PE transpose: kvts[64:128] = V.T
                    def run():
                        kvts = sbuf["kvts"]
                        vt_ps = mp.tile([128, 64], F16, tag="mp",
                                        name="vt_ps")
                        nc.tensor.transpose(vt_ps,
                                            kvts[64:128, 128 * t:128 * t + 128],
                                            identlo_s[64:128, :])
                        blk = q0 // KB + t
                        if fill_phase:
                            # keep the vt->vn evac off the rope-busy DVE so the
                            # mp PSUM ring frees quickly for the next projection
                            nc.scalar.copy(vn[:, 65 * blk:65 * blk + 64], vt_ps)
                        else:
                            nc.vector.tensor_copy(vn[:, 65 * blk:65 * blk + 64],
                                                  vt_ps)
                    return run

                vts = [vt_piece(t) for t in range(qw // 128)]

                def rope(qtraw, kvts):
                    # RoPE: rot = raw*C + swapped*S3. Chunk 0 sits on the
                    # startup critical path: fold the rotate-pair swap into the
                    # S3 muls via the row-swapped table (cssw) with
                    # partition-shifted DVE outputs -- no staging DMA latency.
                    # Later chunks have pipeline slack: stage the swap through
                    # SBUF->SBUF DMAs on the idle gpsimd queue (cs channel 1 is
                    # the plain S3 table).
                    t1 = work.tile([128, QC], F16, tag="t1")
                    t2 = work.tile([128, QC], F16, tag="t2")
                    t3 = work.tile([64, QC], F16, tag="t1")
                    t4 = work.tile([64, QC], F16, tag="t2")
                    rope_body(qtraw, kvts, t1, t2, t3, t4)

                def rope_body(qtraw, kvts, t1, t2, t3, t4):
                    if early:
                        # K-side first: kvts comes off the DVE evac, qtraw off
                        # the scalar engine in parallel
                        nc.vector.tensor_mul(t3[:, 0:qw], kvts[0:64, 0:qw],
                                             cs_s[0:64, 0, ls])
                        nc.vector.tensor_mul(t4[0:32, 0:qw], kvts[32:64, 0:qw],
                                             cssw_s[32:64, ls])
                        nc.vector.tensor_mul(t4[32:64, 0:qw], kvts[0:32, 0:qw],
                                             cssw_s[0:32, ls])
                        nc.vector.tensor_add(kt2[0:64, ls], t3[:, 0:qw],
                                             t4[:, 0:qw])
                        nc.vector.tensor_add(kt2[64:128, ls], t3[:, 0:qw],
                                             t4[:, 0:qw])
                        nc.vector.tensor_mul(t1[:, 0:qw], qtraw[:, 0:qw],
                                             cs_s[:, 0, ls])
                        for (a, b) in ((0, 32), (32, 0), (64, 96), (96, 64)):
                            nc.vector.tensor_mul(t2[a:a + 32, 0:qw],
                                                 qtraw[b:b + 32, 0:qw],
                                                 cssw_s[b:b + 32, ls])
                        nc.vector.tensor_add(qtrope[:, ls], t1[:, 0:qw],
                                             t2[:, 0:qw])
                    else:
                        nc.vector.tensor_mul(t1[:, 0:qw], qtraw[:, 0:qw],
                                             cs_s[:, 0, ls])
                        nc.vector.tensor_mul(t3[:, 0:qw], kvts[0:64, 0:qw],
                                             cs_s[0:64, 0, ls])
                        qts = work.tile([128, QC], F16, tag="qts")
                        for (a, b) in ((0, 32), (32, 0), (64, 96), (96, 64)):
                            nc.gpsimd.dma_start(out=qts[a:a + 32, 0:qw],
                                                in_=qtraw[b:b + 32, 0:qw])
                        kts = work.tile([64, QC], F16, tag="kts")
                        nc.gpsimd.dma_start(out=kts[0:32, 0:qw],
                                            in_=kvts[32:64, 0:qw])
                        nc.gpsimd.dma_start(out=kts[32:64, 0:qw],
                                            in_=kvts[0:32, 0:qw])
                        nc.vector.tensor_mul(t2[:, 0:qw], qts[:, 0:qw],
                                             cs_s[:, 1, ls])
                        nc.vector.tensor_mul(t4[:, 0:qw], kts[:, 0:qw],
                                             cs_s[0:64, 1, ls])
                        nc.vector.tensor_add(qtrope[:, ls], t1[:, 0:qw],
                                             t2[:, 0:qw])
                        # both kt2 halves written (second add = the "dup")
                        nc.vector.tensor_add(kt2[0:64, ls], t3[:, 0:qw],
                                             t4[:, 0:qw])
                        nc.vector.tensor_add(kt2[64:128, ls], t3[:, 0:qw],
                                             t4[:, 0:qw])

                return pieces, tail_dve, vts

            def proj_compute(ci):
                pieces, tail_dve, vts = make_proj_pieces(ci)
                for p in pieces:
                    p()
                tail_dve()
                for v in vts:
                    v()

            def make_chunk(ci):
                q0, qw = BOUNDS[ci]
                qs = slice(q0, q0 + qw)
                d0 = q0 // KB                  # first diagonal key block
                nkb = (q0 + qw) // KB
                state = {}

                def qk(kb):
                    ks = slice(KB * kb, KB * kb + KB)
                    lo = KB * (kb - d0) if kb > d0 else 0
                    qsn = slice(q0 + lo, q0 + qw)
                    st = stp.tile([128, 2, QC], F32, tag="st")
                    nc.tensor.matmul(st[:, 0, lo:qw], kt2[0:64, ks],
                                     qtrope[0:64, qsn], start=True, stop=True)
                    nc.tensor.matmul(st[:, 1, lo:qw], kt2[64:128, ks],
                                     qtrope[64:128, qsn], start=True, stop=True)
                    pt = ptp.tile([128, 2, QC], F16, tag="pt")
                    nc.scalar.activation(pt[:, :, lo:qw], st[:, :, lo:qw],
                                         mybir.ActivationFunctionType.Exp,
                                         scale=0.125)
                    if kb >= d0:
                        nc.vector.tensor_mul(pt[:, 0, lo:lo + KB],
                                             pt[:, 0, lo:lo + KB], tri_s)
                        nc.vector.tensor_mul(pt[:, 1, lo:lo + KB],
                                             pt[:, 1, lo:lo + KB], tri_s)
                    return pt

                def pv(kb, pt, is_first, is_last):
                    if is_first:
                        state["ot0"] = otp.tile([65, QC], F32, tag="ot", name="ot0")
                        state["ot1"] = otp.tile([65, QC], F32, tag="ot", name="ot1")
                    lo = KB * (kb - d0) if kb >= d0 else 0
                    vblk = vn[:, 65 * kb:65 * kb + 65]
                    nc.tensor.matmul(state["ot0"][:, lo:qw], vblk, pt[:, 0, lo:qw],
                                     start=is_first, stop=is_last,
                                     skip_group_check=True)
                    nc.tensor.matmul(state["ot1"][:, lo:qw], vblk, pt[:, 1, lo:qw],
                                     start=is_first, stop=is_last,
                                     skip_group_check=True)

                def finish_a_last():
                    """finish_a for the final chunk, pipelined in halves: all
                    reciprocals first, then per-half broadcast -> normalize so
                    the output projection starts on the first half while the
                    second half's broadcast is still on gpsimd."""
                    rc2 = work.tile([1, 2 * QC], F16, tag="rc2")
                    h = qw // 2
                    with nc.allow_low_precision(reason="softmax denom recip fp16"):
                        nc.vector.reciprocal(rc2[:, 0:qw], state["ot0"][64:65, 0:qw])
                        nc.vector.reciprocal(rc2[:, QC:QC + qw],
                                             state["ot1"][64:65, 0:qw])
                    rbc = work.tile([64, 2 * QC], F16, tag="rbc")
                    otn = work.tile([128, QC], F16, tag="otn")
                    for (a, b) in ((0, h), (h, qw)):
                        nc.gpsimd.partition_broadcast(rbc[:, a:b], rc2[:, a:b])
                        nc.gpsimd.partition_broadcast(rbc[:, QC + a:QC + b],
                                                      rc2[:, QC + a:QC + b])
                        nc.vector.tensor_mul(otn[0:64, a:b],
                                             state["ot0"][0:64, a:b],
                                             rbc[:, a:b])
                        nc.vector.tensor_mul(otn[64:128, a:b],
                                             state["ot1"][0:64, a:b],
                                             rbc[:, QC + a:QC + b])
                    state["otn"] = otn

                def finish_a():
                    # softmax denominators: reciprocal straight off the PSUM
                    # ones-row, then replicate across 64 partitions on gpsimd
                    rc2 = work.tile([1, 2 * QC], F16, tag="rc2")
                    with nc.allow_low_precision(reason="softmax denom recip fp16"):
                        nc.vector.reciprocal(rc2[:, 0:qw], state["ot0"][64:65, 0:qw])
                        nc.vector.reciprocal(rc2[:, QC:QC + qw],
                                             state["ot1"][64:65, 0:qw])
                    rbc = work.tile([64, 2 * QC], F16, tag="rbc")
                    nc.gpsimd.partition_broadcast(rbc[:, 0:qw], rc2[:, 0:qw])
                    nc.gpsimd.partition_broadcast(rbc[:, QC:QC + qw],
                                                  rc2[:, QC:QC + qw])
                    # normalize here (not in finish_b) so these DVE ops sit
                    # ahead of the next chunk's mask ops in the DVE stream --
                    # the next chunk's pv(0) blocks on them via the ot ring
                    otn = work.tile([128, QC], F16, tag="otn")
                    nc.vector.tensor_mul(otn[0:64, 0:qw], state["ot0"][0:64, 0:qw],
                                         rbc[:, 0:qw])
                    nc.vector.tensor_mul(otn[64:128, 0:qw],
                                         state["ot1"][0:64, 0:qw],
                                         rbc[:, QC:QC + qw])
                    state["otn"] = otn

                def wo_pieces():
                    """Eight deferrable output-projection pieces (one matmul +
                    evac each; the last also flushes the staging DMA). Legal
                    any time after this chunk's finish_a; deferred into later
                    chunks' key-block loops as PE filler."""
                    staging = {}

                    def mk(dc):
                        def run(act_evac=False):
                            if "ysb" not in staging:
                                staging["ysb"] = ybp.tile([128, 8, QC], F16,
                                                          tag="ysb",
                                                          name="ysbbig")
                            ysbbig = staging["ysb"]
                            yps = mp.tile([128, QC], F32, tag="mp",
                                          name="yps")
                            nc.tensor.matmul(yps[:, 0:qw], wo01_s[:, dc, :],
                                             state["otn"][:, 0:qw],
                                             start=True, stop=True)
                            if act_evac and dc % 2:
                                # tail context: scalar engine is past its last
                                # exp, split the evac load off the DVE
                                nc.scalar.copy(ysbbig[:, dc, 0:qw],
                                               yps[:, 0:qw])
                            else:
                                nc.vector.tensor_copy(ysbbig[:, dc, 0:qw],
                                                      yps[:, 0:qw])
                            # flush in halves: two 0.5MB DMAs overlap the rest
                            # of the pipeline better than one 1MB DMA at dc7
                            if dc == 3:
                                nc.sync.dma_start(out=yt_r[:, 0:4, qs],
                                                  in_=ysbbig[:, 0:4, 0:qw])
                            elif dc == 7:
                                nc.sync.dma_start(out=yt_r[:, 4:8, qs],
                                                  in_=ysbbig[:, 4:8, 0:qw])
                        return run

                    return [mk(dc) for dc in range(8)]

                def finish_b_last():
                    # final chunk: single-dc PSUMs first and last, whole
                    # score-PSUM tiles (now dead) for the middle pairs; each
                    # projection matmul runs as two half-width pieces so the
                    # first halves start while otn's second half is still
                    # normalizing; alternate DVE/scalar evacuation and
                    # SP/gpsimd DMA queues to shorten the drain tail
                    otn = state["otn"]
                    hw_ = qw // 2
                    yp0 = mp.tile([128, QC], F32, tag="mp", name="yp0")
                    yp1 = mp.tile([128, QC], F32, tag="mp", name="yp1")
                    ypA = stp.tile([128, 2, QC], F32, tag="st")
                    ypB = stp.tile([128, 2, QC], F32, tag="st")
                    plan = [(yp0[:, 0:qw], 0), (yp1[:, 0:qw], 1),
                            (ypA[:, 0, :], 2), (ypA[:, 1, :], 3),
                            (ypB[:, 0, :], 4), (ypB[:, 1, :], 5)]
                    for dst, dc in plan:
                        nc.tensor.matmul(dst[:, 0:hw_], wo01_s[:, dc, :],
                                         otn[:, 0:hw_], start=True, stop=True)
                    yp6 = mp.tile([128, QC], F32, tag="mp", name="yp6")
                    yp7 = mp.tile([128, QC], F32, tag="mp", name="yp7")
                    plan2 = [(yp6[:, 0:qw], 6), (yp7[:, 0:qw], 7)]
                    for dst, dc in plan + plan2:
                        nc.tensor.matmul(dst[:, hw_:qw], wo01_s[:, dc, :],
                                         otn[:, hw_:qw], start=True, stop=True)
                    for dst, dc in plan2:
                        nc.tensor.matmul(dst[:, 0:hw_], wo01_s[:, dc, :],
                                         otn[:, 0:hw_], start=True, stop=True)
                    ysb0 = ylast.tile([128, QC], F16, tag="ysb2")
                    nc.vector.tensor_copy(ysb0, yp0)
                    nc.sync.dma_start(out=yt_r[:, 0, qs], in_=ysb0)
                    ysb1 = ylast.tile([128, QC], F16, tag="ysb2")
                    nc.scalar.copy(ysb1, yp1)
                    nc.gpsimd.dma_start(out=yt_r[:, 1, qs], in_=ysb1)
                    ysbA = ylast.tile([128, 2, QC], F16, tag="ysbp")
                    nc.vector.tensor_copy(ysbA, ypA)
                    nc.sync.dma_start(out=yt_r[:, 2:4, qs], in_=ysbA)
                    ysbB = ylast.tile([128, 2, QC], F16, tag="ysbp")
                    nc.scalar.copy(ysbB, ypB)
                    nc.gpsimd.dma_start(out=yt_r[:, 4:6, qs], in_=ysbB)
                    ysb6 = ylast.tile([128, QC], F16, tag="ysb2")
                    nc.vector.tensor_copy(ysb6, yp6)
                    nc.sync.dma_start(out=yt_r[:, 6, qs], in_=ysb6)
                    ysb7 = ylast.tile([128, QC], F16, tag="ysb2")
                    nc.scalar.copy(ysb7, yp7)
                    nc.gpsimd.dma_start(out=yt_r[:, 7, qs], in_=ysb7)

                return (nkb, qk, pv, finish_a, wo_pieces, finish_b_last,
                        finish_a_last)

            proj_dma(0)
            proj_compute(0)
            proj_dma(1)
            # chunk 1's projection emitted before chunk 0's attention: fills
            # the PE while chunk 0's RoPE chain runs on DVE/scalar
            proj_compute(1)
            proj_dma(2)
            load_late_consts()
            proj_dma(3)
            # --- global PE-filler scheduler state ---
            # wo(ci) is deferred exactly WO_DELAY chunks (bounded by the otn
            # ring depth) so the late, projection-free chunks still get one
            # ~213ns filler matmul per key block against the scalar engine's
            # ~1038ns/kb exp cadence (PE qk+pv alone is only ~852ns/kb).
            WO_DELAY = 4
            wo_q = []              # ready filler pieces (FIFO)
            wo_pending = {}        # ci -> its 8 wo pieces, not yet released
            prev = None
            for ci in range(NCH):
                q0, qw = BOUNDS[ci]
                d0 = q0 // KB
                (nkb, qk, pv, finish_a, wo_pieces, finish_b_last,
                 finish_a_last) = make_chunk(ci)
                diags0 = [kb for kb in range(d0, nkb) if kb != 0]
                second = diags0[0] if diags0 else 1
                last = ci == NCH - 1
                # during pipeline fill the next chunk's projection goes ahead
                # of this chunk's (rope-gated) first qk in the in-order PE
                # stream; in steady state its 16 matmuls become fillers
                if 1 <= ci <= 2 and ci < NCH - 1:
                    proj_compute(ci + 1)
                    pieces, ptail, vts = [], None, []
                elif 3 <= ci < NCH - 1:
                    pieces, ptail, vts = make_proj_pieces(ci + 1)
                else:
                    pieces, ptail, vts = [], None, []
                pts = {}
                pts[0] = qk(0)
                # previous chunk's finish_a before qk(second): its reciprocals
                # precede the new chunk's first diagonal masks in the in-order
                # DVE queue (they are ready first; the masks wait on exp)
                if prev is not None:
                    prev()
                if nkb > 1:
                    pts[second] = qk(second)
                if ci + 4 < NCH:
                    proj_dma(ci + 4)
                # release wo work whose deferral window ends at this chunk
                rel = ci - WO_DELAY
                if rel in wo_pending:
                    wo_q.extend(wo_pending.pop(rel))
                if last:
                    for cj in sorted(wo_pending):
                        wo_q.extend(wo_pending.pop(cj))
                # diagonal k-blocks early: their masks leave the boundary's
                # critical path; block 0 stays first (full-width start=True)
                diags = [kb for kb in range(d0, nkb) if kb != 0]
                rest = [kb for kb in range(1, d0)]
                order = [0] + diags + rest
                vt_at = None
                for i, kb in enumerate(order):
                    if i + 2 < nkb:
                        pts[order[i + 2]] = qk(order[i + 2])
                    # one filler piece per key block, emitted before pv so a
                    # dependency-stalled pv doesn't idle the in-order PE
                    emitted = 0
                    while pieces and len(pieces) > max(0, nkb - 4 - i):
                        pieces.pop(0)()
                        emitted += 1
                    if pieces and emitted == 0:
                        pieces.pop(0)()
                        emitted = 1
                    if not pieces and ptail is not None:
                        ptail()
                        ptail = None
                        vt_at = i + 2
                    if vt_at is not None and i >= vt_at and vts:
                        vts.pop(0)()
                        emitted = 1
                    if last:
                        # front-load ~1.5 wo pieces/slot (DVE evac budget),
                        # none in the last slots, so the drain window has no
                        # trailing evac/DMA queue
                        if i < nkb - 6:
                            for _ in range(2 if i % 2 == 0 else 1):
                                if wo_q:
                                    wo_q.pop(0)()
                    elif emitted == 0 and wo_q:
                        wo_q.pop(0)()
                    pv(kb, pts.pop(kb), i == 0, i == nkb - 1)
                # drain any leftover projection work before the boundary
                while pieces:
                    pieces.pop(0)()
                if ptail is not None:
                    ptail()
                while vts:
                    vts.pop(0)()
                if last:
                    finish_a_last()
                    while wo_q:
                        wo_q.pop(0)(True)
                    finish_b_last()
                else:
                    prev = finish_a
                    wo_pending[ci] = wo_pieces()

    nc.finalize()
    return nc


def prep_inputs(x, Wq, Wk, Wv, Wo, token_positions, L=4096):
    """Host-side sharding + layout prep. Returns per-core input maps."""
    x = np.asarray(x, dtype=np.float32)
    Wq = np.asarray(Wq, dtype=np.float32)
    Wk = np.asarray(Wk, dtype=np.float32)
    Wv = np.asarray(Wv, dtype=np.float32)
    Wo = np.asarray(Wo, dtype=np.float32)
    pos = np.asarray(token_positions)[0].astype(np.float64)

    xt = np.ascontiguousarray(x[0].T).astype(np.float16)   # [D, L]
    i = np.arange(HEAD_DIM // 2, dtype=np.float64)
    freq = THETA ** (-2.0 * i / HEAD_DIM)                  # [32]
    ang = pos[:, None] * freq[None, :]                     # [L, 32]
    cos = np.cos(ang).T
    sin = np.sin(ang).T
    c64 = np.concatenate([cos, cos], axis=0)               # [64, L]
    s64 = np.concatenate([-sin, sin], axis=0)
    ctab = np.concatenate([c64, c64], axis=0)              # [128, L]
    s3tab = np.concatenate([s64, s64], axis=0)
    swapperm = np.concatenate([np.arange(32, 64), np.arange(0, 32),
                               np.arange(96, 128), np.arange(64, 96)])
    s3sw = s3tab[swapperm]   # row-swapped S3 for the fused early-chunk path
    cs2 = np.ascontiguousarray(
        np.stack([ctab, s3tab], axis=1)).astype(np.float16)   # [128, 2, L]

    perm = np.concatenate([np.arange(0, 64, 2), np.arange(1, 64, 2)])
    tri = (np.arange(128)[None, :] >= np.arange(128)[:, None]).astype(np.float16)
    identlo = np.zeros((128, 64), dtype=np.float16)
    identlo[np.arange(128), np.arange(128) % 64] = 1.0
    auxm = np.concatenate([tri, identlo, s3sw[:, 0:QC]], axis=1)
    auxm = np.ascontiguousarray(auxm).astype(np.float16)   # [128, 192+QC]

    in_maps = []
    for c in range(N_CORES):
        h0, h1, g = 2 * c, 2 * c + 1, c // 2
        qrows = np.concatenate([64 * h0 + perm, 64 * h1 + perm])
        # weight layouts pre-arranged as [p, dc, m] so the load DMA is one
        # contiguous 2KB-per-partition transfer
        wqt = np.ascontiguousarray(
            Wq[qrows, :].T.reshape(8, 128, 128).transpose(1, 0, 2)
        ).astype(np.float16)
        kv = np.concatenate([Wk[64 * g + perm, :], Wv[64 * g:64 * g + 64, :]], axis=0)
        wkvt = np.ascontiguousarray(
            kv.T.reshape(8, 128, 128).transpose(1, 0, 2)).astype(np.float16)
        attnrows = np.concatenate([np.arange(64 * h0, 64 * h0 + 64),
                                   np.arange(64 * h1, 64 * h1 + 64)])
        wo01 = np.ascontiguousarray(
            Wo[:, attnrows].T.reshape(128, 8, 128)).astype(np.float16)
        in_maps.append(dict(xt=xt, wqt=wqt, wkvt=wkvt, wo01=wo01,
                            cs2=cs2, aux=auxm))
    return in_maps


_NC_CACHE = {}


def _get_nc(L=4096):
    if L not in _NC_CACHE:
        _NC_CACHE[L] = build_kernel(L)
    return _NC_CACHE[L]


def kernel(x, Wq, Wk, Wv, Wo, token_positions):
    B, L, D = np.asarray(x).shape
    nc = _get_nc(L)
    in_maps = prep_inputs(x, Wq, Wk, Wv, Wo, token_positions, L=L)
    res = run_bass_kernel_spmd(nc, in_maps, list(range(N_CORES)))
    y = np.zeros((D_MODEL, L), dtype=np.float32)
    for r in res.results:
        y += r["yt"].astype(np.float32)
    return np.ascontiguousarray(y.T)[None].astype(np.float32)


# revision 39
# speedup vs baseline: 1.0072x; 1.0072x over previous
"""Trainium2 Bass kernel: GQA multi-head self-attention (B=1, L=4096, D=1024,
16 Q heads, 4 KV heads, head_dim 64, interleaved RoPE, causal softmax).

Sharding: 2 query heads + their (shared) KV head per core, 8 cores.
Each core computes a full-shape partial output Y_c.T = (attn_c @ Wo_c.T).T
(Megatron row-parallel style); the host sums the 8 partials.

Device-side design (per core):
  - x is fed pre-transposed (xT [D, L], fp16) so projection matmuls stream
    natural SBUF tiles; matmul operands are fp16 (1 cycle/row on the PE),
    accumulation stays fp32 in PSUM.
  - Q.T/K.T are produced in a "half-split" head-dim order (even dims then odd
    dims per head, via host-permuted weight rows) so RoPE's rotate-pair becomes
    a 32-partition block swap, done with SBUF->SBUF DMAs.
  - Attention runs in the S.T = K @ Q.T orientation: scores land in PSUM as
    [k=128, 2, q] tiles (both heads in one tile), exp runs on the scalar
    engine straight out of PSUM, and PV uses [V | ones] as the stationary
    operand so softmax denominators come out as row 64 of the PV accumulator
    for free. Diagonal key-blocks compute only the causally live query columns
    (matmul, exp and PV all narrowed).
  - Softmax normalization: DVE reciprocal straight off the PSUM denominator
    row, gpsimd partition_broadcast to replicate it across 64 partitions, one
    fused [128, q] attention-out tile so the output projection is 8 single
    (contraction-128) matmuls per chunk.
  - No max-subtraction pass: scores are O(1) here, exp cannot overflow, and
    softmax is shift-invariant so the result matches the reference.
  - Chunks have variable width: the first 512 columns are processed as two
    256-wide chunks so the startup pipeline (x DMA -> proj -> RoPE -> QK)
    fills in half the time; early-chunk PSUM evacuation runs on the (then
    idle) scalar engine to shorten the serial DVE RoPE chain.
  - Emission is software-pipelined: QK^T/exp run two key-blocks ahead of PV,
    and each chunk's normalize + output projection is deferred until the next
    chunk's first key-blocks are in flight, its 8 output-projection matmuls
    spread one-per-key-block so the PSUM ring never blocks the in-order PE
    stream. Non-final chunks store the projected output in one [128, 8, qw]
    staging tile and issue a single batched DMA; the final chunk uses paired
    PSUM tiles with alternating DVE/scalar evacuation and SP/gpsimd DMA
    queues to shorten the drain tail.
"""

import sys

for _p in ("/opt/trn_rl_repo",):
    if _p not in sys.path:
        sys.path.insert(0, _p)

import numpy as np

import concourse.bacc as bacc
import concourse.mybir as mybir
import concourse.tile as tile
from concourse.bass_utils import run_bass_kernel_spmd

F32 = mybir.dt.float32
F16 = mybir.dt.float16

D_MODEL = 1024
NUM_HEADS = 16
NUM_KV_HEADS = 4
HEAD_DIM = 64
THETA = 10000.0
N_CORES = 8
QC = 512          # max query chunk (free dim of S.T tiles per head)
KB = 128          # key block (partition dim of S.T tiles)


def chunk_bounds(L):
    """(q0, qw) per chunk: first 512 cols as two 256-wide chunks, then 512s."""
    out = [(0, 512)]
    q = 512
    while q < L:
        out.append((q, 512))
        q += 512
    return out


def build_kernel(L=4096):
    """One-core SPMD program. Handles its 2 query heads + 1 shared KV head."""
    nc = bacc.Bacc(None, target_bir_lowering=False)
    NT = L // KB          # number of 128-row key blocks / V tiles
    BOUNDS = chunk_bounds(L)
    NCH = len(BOUNDS)

    xt = nc.dram_tensor("xt", [D_MODEL, L], F16, kind="ExternalInput")
    wqt = nc.dram_tensor("wqt", [128, 8, 128], F16, kind="ExternalInput")
    wkvt = nc.dram_tensor("wkvt", [128, 8, 128], F16, kind="ExternalInput")
    wo01 = nc.dram_tensor("wo01", [128, 8, 128], F16, kind="ExternalInput")
    cs2 = nc.dram_tensor("cs2", [128, 2, L], F16, kind="ExternalInput")
    # aux packs [tri | identlo | cs0-swapped-S] so startup needs one small DMA
    aux = nc.dram_tensor("aux", [128, 192 + QC], F16, kind="ExternalInput")
    yt = nc.dram_tensor("yt", [D_MODEL, L], F16, kind="ExternalOutput")

    xt_r = xt.rearrange("(dc p) l -> p dc l", p=128)      # [128, 8, L]
    yt_r = yt.rearrange("(dc p) l -> p dc l", p=128)      # [128, 8, L]

    with tile.TileContext(nc) as tc:
        with (
            tc.tile_pool(name="consts", bufs=1) as consts,
            tc.tile_pool(name="big", bufs=1) as big,
            tc.tile_pool(name="xin", bufs=4) as xin,
            tc.tile_pool(name="work", bufs=5) as work,
            tc.tile_pool(name="ybp", bufs=2) as ybp,
            tc.tile_pool(name="ylast", bufs=8) as ylast,
            tc.tile_pool(name="ptp", bufs=14) as ptp,
            tc.tile_pool(name="stp", bufs=2, space="PSUM") as stp,
            tc.tile_pool(name="otp", bufs=2, space="PSUM") as otp,
            tc.tile_pool(name="mp", bufs=2, space="PSUM") as mp,
        ):
            # ---- constants in SBUF ----
            wqt_s = consts.tile([128, 8, 128], F16, tag="wqt")
            wkvt_s = consts.tile([128, 8, 128], F16, tag="wkvt")
            wo01_s = consts.tile([128, 8, 128], F16, tag="wo01")
            cs_s = consts.tile([128, 2, L], F16, tag="cs")
            aux_s = consts.tile([128, 192 + QC], F16, tag="aux")
            tri_s = aux_s[:, 0:128]
            identlo_s = aux_s[:, 128:192]
            cssw_s = aux_s[:, 192:192 + QC]

            def load_late_consts():
                nc.sync.dma_start(out=wo01_s, in_=wo01[:, :, :])

            # ---- persistent per-core activations ----
            qtrope = big.tile([128, L], F16, tag="qtrope")      # [2*64 halfsplit d, L]
            kt2 = big.tile([128, L], F16, tag="kt2")            # K.T duplicated twice
            vn = big.tile([128, NT * 65], F16, tag="vn")        # [V | 1] blocks
            nc.gpsimd.memset(vn[:, 64::65], 1.0)                # just the ones columns

            xtiles = {}

            def proj_dma(ci):
                q0, qw = BOUNDS[ci]
                ls = slice(q0, q0 + qw)
                xbig = xin.tile([128, 8, QC], F16, tag="xt")
                if ci == 0:
                    # startup ordering: DMAs staged in matmul-consumption order
                    # (q dc0-3, kv dc0-3, q dc4-7, kv dc4-7) at fine grain so
                    # the first projection matmul starts after ~0.3MB
                    nc.sync.dma_start(out=wqt_s[:, 0:4, :], in_=wqt[:, 0:4, :])
                    nc.sync.dma_start(out=xbig[:, 0:4, 0:qw], in_=xt_r[:, 0:4, ls])
                    nc.sync.dma_start(out=wkvt_s[:, 0:4, :], in_=wkvt[:, 0:4, :])
                    nc.sync.dma_start(out=wqt_s[:, 4:8, :], in_=wqt[:, 4:8, :])
                    nc.sync.dma_start(out=xbig[:, 4:8, 0:qw], in_=xt_r[:, 4:8, ls])
                    nc.sync.dma_start(out=wkvt_s[:, 4:8, :], in_=wkvt[:, 4:8, :])
                    nc.sync.dma_start(out=cs_s[:, 0, ls], in_=cs2[:, 0, ls])
                    nc.sync.dma_start(out=aux_s, in_=aux[:, :])
                else:
                    nc.sync.dma_start(out=xbig[:, :, 0:qw], in_=xt_r[:, :, ls])
                    nc.sync.dma_start(out=cs_s[:, :, ls], in_=cs2[:, :, ls])
                xtiles[ci] = xbig

            def make_proj_pieces(ci):
                q0, qw = BOUNDS[ci]
                ls = slice(q0, q0 + qw)
                xbig = xtiles.pop(ci)
                psum = {}

                def mm_piece(kind, dc):
                    def run():
                        if not psum:
                            psum["qt"] = mp.tile([128, QC], F32, tag="mp",
                                                  name="qt_ps")
                            psum["kv"] = mp.tile([128, QC], F32, tag="mp",
                                                  name="kvt_ps")
                        ps = psum["qt"] if kind == "q" else psum["kv"]
                        w = wqt_s if kind == "q" else wkvt_s
                        nc.tensor.matmul(ps[:, 0:qw], w[:, dc, :],
                                         xbig[:, dc, 0:qw],
                                         start=(dc == 0), stop=(dc == 7))
                    return run

                # half-interleaved so the low x half can be consumed while the
                # high half's DMA is still in flight (matters for chunk 0)
                pieces = ([mm_piece("q", dc) for dc in range(4)]
                          + [mm_piece("kv", dc) for dc in range(4)]
                          + [mm_piece("q", dc) for dc in range(4, 8)]
                          + [mm_piece("kv", dc) for dc in range(4, 8)])

                early = q0 + qw <= QC
                fill_phase = q0 + qw <= 3 * QC
                sbuf = {}

                def tail_dve():
                    # evacuate PSUM (fp32 -> fp16); early chunks route the
                    # Q-side evac to the (idle) scalar engine so the DVE RoPE
                    # chain starts sooner
                    qtraw = work.tile([128, QC], F16, tag="qtraw")
                    kvts = work.tile([128, QC], F16, tag="kvts")
                    sbuf["kvts"] = kvts
                    nc.vector.tensor_copy(kvts[:, 0:qw], psum["kv"][:, 0:qw])
                    if fill_phase:
                        nc.scalar.copy(qtraw[:, 0:qw], psum["qt"][:, 0:qw])
                    else:
                        nc.vector.tensor_copy(qtraw[:, 0:qw], psum["qt"][:, 0:qw])
                    rope(qtraw, kvts)

                def vt_piece(t):
                    # V natural layout via PE transpose: kvts[64:128] = V.T
                    def run():
                        kvts = sbuf["kvts"]
                        vt_ps = mp.tile([128, 64], F16, tag="mp",
                                        name="vt_ps")
                        nc.tensor.transpose(vt_ps,
                                            kvts[64:128, 128 * t:128 * t + 128],
                                            identlo_s[64:128, :])
                        blk = q0 // KB + t
                        if fill_phase:
                            # keep the vt->vn evac off the rope-busy DVE so the
                            # mp PSUM ring frees quickly for the next projection
                            nc.scalar.copy(vn[:, 65 * blk:65 * blk + 64], vt_ps)
                        else:
                            nc.vector.tensor_copy(vn[:, 65 * blk:65 * blk + 64],
                                                  vt_ps)
                    return run

                vts = [vt_piece(t) for t in range(qw // 128)]

                def rope(qtraw, kvts):
                    # RoPE: rot = raw*C + swapped*S3. Chunk 0 sits on the
                    # startup critical path: fold the rotate-pair swap into the
                    # S3 muls via the row-swapped table (cssw) with
                    # partition-shifted DVE outputs -- no staging DMA latency.
                    # Later chunks have pipeline slack: stage the swap through
                    # SBUF->SBUF DMAs on the idle gpsimd queue (cs channel 1 is
                    # the plain S3 table).
                    t1 = work.tile([128, QC], F16, tag="t1")
                    t2 = work.tile([128, QC], F16, tag="t2")
                    t3 = work.tile([64, QC], F16, tag="t1")
                    t4 = work.tile([64, QC], F16, tag="t2")
                    rope_body(qtraw, kvts, t1, t2, t3, t4)

                def rope_body(qtraw, kvts, t1, t2, t3, t4):
                    if early:
                        # K-side first: kvts comes off the DVE evac, qtraw off
                        # the scalar engine in parallel
                        nc.vector.tensor_mul(t3[:, 0:qw], kvts[0:64, 0:qw],
                                             cs_s[0:64, 0, ls])
                        nc.vector.tensor_mul(t4[0:32, 0:qw], kvts[32:64, 0:qw],
                                             cssw_s[32:64, ls])
                        nc.vector.tensor_mul(t4[32:64, 0:qw], kvts[0:32, 0:qw],
                                             cssw_s[0:32, ls])
                        nc.vector.tensor_add(kt2[0:64, ls], t3[:, 0:qw],
                                             t4[:, 0:qw])
                        nc.vector.tensor_add(kt2[64:128, ls], t3[:, 0:qw],
                                             t4[:, 0:qw])
                        nc.vector.tensor_mul(t1[:, 0:qw], qtraw[:, 0:qw],
                                             cs_s[:, 0, ls])
                        for (a, b) in ((0, 32), (32, 0), (64, 96), (96, 64)):
                            nc.vector.tensor_mul(t2[a:a + 32, 0:qw],
                                                 qtraw[b:b + 32, 0:qw],
                                                 cssw_s[b:b + 32, ls])
                        nc.vector.tensor_add(qtrope[:, ls], t1[:, 0:qw],
                                             t2[:, 0:qw])
                    else:
                        nc.vector.tensor_mul(t1[:, 0:qw], qtraw[:, 0:qw],
                                             cs_s[:, 0, ls])
                        nc.vector.tensor_mul(t3[:, 0:qw], kvts[0:64, 0:qw],
                                             cs_s[0:64, 0, ls])
                        qts = work.tile([128, QC], F16, tag="qts")
                        for (a, b) in ((0, 32), (32, 0), (64, 96), (96, 64)):
                            nc.gpsimd.dma_start(out=qts[a:a + 32, 0:qw],
                                                in_=qtraw[b:b + 32, 0:qw])
                        kts = work.tile([64, QC], F16, tag="kts")
                        nc.gpsimd.dma_start(out=kts[0:32, 0:qw],
                                            in_=kvts[32:64, 0:qw])
                        nc.gpsimd.dma_start(out=kts[32:64, 0:qw],
                                            in_=kvts[0:32, 0:qw])
                        nc.vector.tensor_mul(t2[:, 0:qw], qts[:, 0:qw],
                                             cs_s[:, 1, ls])
                        nc.vector.tensor_mul(t4[:, 0:qw], kts[:, 0:qw],
                                             cs_s[0:64, 1, ls])
                        nc.vector.tensor_add(qtrope[:, ls], t1[:, 0:qw],
                                             t2[:, 0:qw])
                        # both kt2 halves written (second add = the "dup")
                        nc.vector.tensor_add(kt2[0:64, ls], t3[:, 0:qw],
                                             t4[:, 0:qw])
                        nc.vector.tensor_add(kt2[64:128, ls], t3[:, 0:qw],
                                             t4[:, 0:qw])

                return pieces, tail_dve, vts

            def proj_compute(ci):
                pieces, tail_dve, vts = make_proj_pieces(ci)
                for p in pieces:
                    p()
                tail_dve()
                for v in vts:
                    v()

            def make_chunk(ci):
                q0, qw = BOUNDS[ci]
                qs = slice(q0, q0 + qw)
                d0 = q0 // KB                  # first diagonal key block
                nkb = (q0 + qw) // KB
                state = {}

                def qk(kb):
                    ks = slice(KB * kb, KB * kb + KB)
                    lo = KB * (kb - d0) if kb > d0 else 0
                    qsn = slice(q0 + lo, q0 + qw)
                    st = stp.tile([128, 2, QC], F32, tag="st")
                    nc.tensor.matmul(st[:, 0, lo:qw], kt2[0:64, ks],
                                     qtrope[0:64, qsn], start=True, stop=True)
                    nc.tensor.matmul(st[:, 1, lo:qw], kt2[64:128, ks],
                                     qtrope[64:128, qsn], start=True, stop=True)
                    pt = ptp.tile([128, 2, QC], F16, tag="pt")
                    nc.scalar.activation(pt[:, :, lo:qw], st[:, :, lo:qw],
                                         mybir.ActivationFunctionType.Exp,
                                         scale=0.125)
                    if kb >= d0:
                        nc.vector.tensor_mul(pt[:, 0, lo:lo + KB],
                                             pt[:, 0, lo:lo + KB], tri_s)
                        nc.vector.tensor_mul(pt[:, 1, lo:lo + KB],
                                             pt[:, 1, lo:lo + KB], tri_s)
                    return pt

                def pv(kb, pt, is_first, is_last):
                    if is_first:
                        state["ot0"] = otp.tile([65, QC], F32, tag="ot", name="ot0")
                        state["ot1"] = otp.tile([65, QC], F32, tag="ot", name="ot1")
                    lo = KB * (kb - d0) if kb >= d0 else 0
                    vblk = vn[:, 65 * kb:65 * kb + 65]
                    nc.tensor.matmul(state["ot0"][:, lo:qw], vblk, pt[:, 0, lo:qw],
                                     start=is_first, stop=is_last,
                                     skip_group_check=True)
                    nc.tensor.matmul(state["ot1"][:, lo:qw], vblk, pt[:, 1, lo:qw],
                                     start=is_first, stop=is_last,
                                     skip_group_check=True)

                def finish_a_last():
                    """finish_a for the final chunk, pipelined in halves: all
                    reciprocals first, then per-half broadcast -> normalize so
                    the output projection starts on the first half while the
                    second half's broadcast is still on gpsimd."""
                    rc2 = work.tile([1, 2 * QC], F16, tag="rc2")
                    h = qw // 2
                    with nc.allow_low_precision(reason="softmax denom recip fp16"):
                        nc.vector.reciprocal(rc2[:, 0:qw], state["ot0"][64:65, 0:qw])
                        nc.vector.reciprocal(rc2[:, QC:QC + qw],
                                             state["ot1"][64:65, 0:qw])
                    rbc = work.tile([64, 2 * QC], F16, tag="rbc")
                    otn = work.tile([128, QC], F16, tag="otn")
                    for (a, b) in ((0, h), (h, qw)):
                        nc.gpsimd.partition_broadcast(rbc[:, a:b], rc2[:, a:b])
                        nc.gpsimd.partition_broadcast(rbc[:, QC + a:QC + b],
                                                      rc2[:, QC + a:QC + b])
                        nc.vector.tensor_mul(otn[0:64, a:b],
                                             state["ot0"][0:64, a:b],
                                             rbc[:, a:b])
                        nc.vector.tensor_mul(otn[64:128, a:b],
                                             state["ot1"][0:64, a:b],
                                             rbc[:, QC + a:QC + b])
                    state["otn"] = otn

                def finish_a():
                    # softmax denominators: reciprocal straight off the PSUM
                    # ones-row, then replicate across 64 partitions on gpsimd
                    rc2 = work.tile([1, 2 * QC], F16, tag="rc2")
                    with nc.allow_low_precision(reason="softmax denom recip fp16"):
                        nc.vector.reciprocal(rc2[:, 0:qw], state["ot0"][64:65, 0:qw])
                        nc.vector.reciprocal(rc2[:, QC:QC + qw],
                                             state["ot1"][64:65, 0:qw])
                    rbc = work.tile([64, 2 * QC], F16, tag="rbc")
                    nc.gpsimd.partition_broadcast(rbc[:, 0:qw], rc2[:, 0:qw])
                    nc.gpsimd.partition_broadcast(rbc[:, QC:QC + qw],
                                                  rc2[:, QC:QC + qw])
                    # normalize here (not in finish_b) so these DVE ops sit
                    # ahead of the next chunk's mask ops in the DVE stream --
                    # the next chunk's pv(0) blocks on them via the ot ring
                    otn = work.tile([128, QC], F16, tag="otn")
                    nc.vector.tensor_mul(otn[0:64, 0:qw], state["ot0"][0:64, 0:qw],
                                         rbc[:, 0:qw])
                    nc.vector.tensor_mul(otn[64:128, 0:qw],
                                         state["ot1"][0:64, 0:qw],
                                         rbc[:, QC:QC + qw])
                    state["otn"] = otn

                def wo_pieces():
                    """Eight deferrable output-projection pieces (one matmul +
                    evac each; the last also flushes the staging DMA). Legal
                    any time after this chunk's finish_a; deferred into later
                    chunks' key-block loops as PE filler."""
                    staging = {}

                    def mk(dc):
                        def run(act_evac=False):
                            if "ysb" not in staging:
                                staging["ysb"] = ybp.tile([128, 8, QC], F16,
                                                          tag="ysb",
                                                          name="ysbbig")
                            ysbbig = staging["ysb"]
                            yps = mp.tile([128, QC], F32, tag="mp",
                                          name="yps")
                            nc.tensor.matmul(yps[:, 0:qw], wo01_s[:, dc, :],
                                             state["otn"][:, 0:qw],
                                             start=True, stop=True)
                            if act_evac and dc % 2:
                                # tail context: scalar engine is past its last
                                # exp, split the evac load off the DVE
                                nc.scalar.copy(ysbbig[:, dc, 0:qw],
                                               yps[:, 0:qw])
                            else:
                                nc.vector.tensor_copy(ysbbig[:, dc, 0:qw],
                                                      yps[:, 0:qw])
                            # flush in halves: two 0.5MB DMAs overlap the rest
                            # of the pipeline better than one 1MB DMA at dc7
                            if dc == 3:
                                nc.sync.dma_start(out=yt_r[:, 0:4, qs],
                                                  in_=ysbbig[:, 0:4, 0:qw])
                            elif dc == 7:
                                nc.sync.dma_start(out=yt_r[:, 4:8, qs],
                                                  in_=ysbbig[:, 4:8, 0:qw])
                        return run

                    return [mk(dc) for dc in range(8)]

                def finish_b_last():
                    # final chunk: single-dc PSUMs first and last, whole
                    # score-PSUM tiles (now dead) for the middle pairs; each
                    # projection matmul runs as two half-width pieces so the
                    # first halves start while otn's second half is still
                    # normalizing; alternate DVE/scalar evacuation and
                    # SP/gpsimd DMA queues to shorten the drain tail
                    otn = state["otn"]
                    hw_ = qw // 2
                    yp0 = mp.tile([128, QC], F32, tag="mp", name="yp0")
                    yp1 = mp.tile([128, QC], F32, tag="mp", name="yp1")
                    ypA = stp.tile([128, 2, QC], F32, tag="st")
                    ypB = stp.tile([128, 2, QC], F32, tag="st")
                    plan = [(yp0[:, 0:qw], 0), (yp1[:, 0:qw], 1),
                            (ypA[:, 0, :], 2), (ypA[:, 1, :], 3),
                            (ypB[:, 0, :], 4), (ypB[:, 1, :], 5)]
                    for dst, dc in plan:
                        nc.tensor.matmul(dst[:, 0:hw_], wo01_s[:, dc, :],
                                         otn[:, 0:hw_], start=True, stop=True)
                    yp6 = mp.tile([128, QC], F32, tag="mp", name="yp6")
                    yp7 = mp.tile([128, QC], F32, tag="mp", name="yp7")
                    plan2 = [(yp6[:, 0:qw], 6), (yp7[:, 0:qw], 7)]
                    for dst, dc in plan + plan2:
                        nc.tensor.matmul(dst[:, hw_:qw], wo01_s[:, dc, :],
                                         otn[:, hw_:qw], start=True, stop=True)
                    for dst, dc in plan2:
                        nc.tensor.matmul(dst[:, 0:hw_], wo01_s[:, dc, :],
                                         otn[:, 0:hw_], start=True, stop=True)
                    ysb0 = ylast.tile([128, QC], F16, tag="ysb2")
                    nc.vector.tensor_copy(ysb0, yp0)
                    nc.sync.dma_start(out=yt_r[:, 0, qs], in_=ysb0)
                    ysb1 = ylast.tile([128, QC], F16, tag="ysb2")
                    nc.scalar.copy(ysb1, yp1)
                    nc.gpsimd.dma_start(out=yt_r[:, 1, qs], in_=ysb1)
                    ysbA = ylast.tile([128, 2, QC], F16, tag="ysbp")
                    nc.vector.tensor_copy(ysbA, ypA)
                    nc.sync.dma_start(out=yt_r[:, 2:4, qs], in_=ysbA)
                    ysbB = ylast.tile([128, 2, QC], F16, tag="ysbp")
                    nc.scalar.copy(ysbB, ypB)
                    nc.gpsimd.dma_start(out=yt_r[:, 4:6, qs], in_=ysbB)
                    ysb6 = ylast.tile([128, QC], F16, tag="ysb2")
                    nc.vector.tensor_copy(ysb6, yp6)
                    nc.sync.dma_start(out=yt_r[:, 6, qs], in_=ysb6)
                    ysb7 = ylast.tile([128, QC], F16, tag="ysb2")
                    nc.scalar.copy(ysb7, yp7)
                    nc.gpsimd.dma_start(out=yt_r[:, 7, qs], in_=ysb7)

                return (nkb, qk, pv, finish_a, wo_pieces, finish_b_last,
                        finish_a_last)

            proj_dma(0)
            proj_compute(0)
            proj_dma(1)
            # chunk 1's projection emitted before chunk 0's attention: fills
            # the PE while chunk 0's RoPE chain runs on DVE/scalar
            proj_compute(1)
            proj_dma(2)
            load_late_consts()
            proj_dma(3)
            # --- global PE-filler scheduler state ---
            # wo(ci) is deferred exactly WO_DELAY chunks (bounded by the otn
            # ring depth) so the late, projection-free chunks still get one
            # ~213ns filler matmul per key block against the scalar engine's
            # ~1038ns/kb exp cadence (PE qk+pv alone is only ~852ns/kb).
            WO_DELAY = 4
            wo_q = []              # ready filler pieces (FIFO)
            wo_pending = {}        # ci -> its 8 wo pieces, not yet released
            prev = None
            for ci in range(NCH):
                q0, qw = BOUNDS[ci]
                d0 = q0 // KB
                (nkb, qk, pv, finish_a, wo_pieces, finish_b_last,
                 finish_a_last) = make_chunk(ci)
                diags0 = [kb for kb in range(d0, nkb) if kb != 0]
                second = diags0[0] if diags0 else 1
                last = ci == NCH - 1
                # during pipeline fill the next chunk's projection goes ahead
                # of this chunk's (rope-gated) first qk in the in-order PE
                # stream; in steady state its 16 matmuls become fillers
                if 1 <= ci <= 2 and ci < NCH - 1:
                    proj_compute(ci + 1)
                    pieces, ptail, vts = [], None, []
                elif 3 <= ci < NCH - 1:
                    pieces, ptail, vts = make_proj_pieces(ci + 1)
                else:
                    pieces, ptail, vts = [], None, []
                pts = {}
                pts[0] = qk(0)
                if nkb > 1:
                    pts[second] = qk(second)
                if prev is not None:
                    prev()
                if ci + 4 < NCH:
                    proj_dma(ci + 4)
                # release wo work whose deferral window ends at this chunk
                rel = ci - WO_DELAY
                if rel in wo_pending:
                    wo_q.extend(wo_pending.pop(rel))
                if last:
                    for cj in sorted(wo_pending):
                        wo_q.extend(wo_pending.pop(cj))
                # diagonal k-blocks early: their masks leave the boundary's
                # critical path; block 0 stays first (full-width start=True)
                diags = [kb for kb in range(d0, nkb) if kb != 0]
                rest = [kb for kb in range(1, d0)]
                order = [0] + diags + rest
                vt_at = None
                for i, kb in enumerate(order):
                    if i + 2 < nkb:
                        pts[order[i + 2]] = qk(order[i + 2])
                    # one filler piece per key block, emitted before pv so a
                    # dependency-stalled pv doesn't idle the in-order PE
                    emitted = 0
                    while pieces and len(pieces) > max(0, nkb - 4 - i):
                        pieces.pop(0)()
                        emitted += 1
                    if pieces and emitted == 0:
                        pieces.pop(0)()
                        emitted = 1
                    if not pieces and ptail is not None:
                        ptail()
                        ptail = None
                        # +4: the transposes read kvts via the DVE evac in
                        # ptail; give it ~2 key blocks of pipeline slack so
                        # the in-order PE doesn't stall on them
                        vt_at = i + 4
                    if vt_at is not None and i >= vt_at and vts:
                        vts.pop(0)()
                    if last:
                        # front-load ~1.5 wo pieces/slot (DVE evac budget),
                        # none in the last slots, so the drain window has no
                        # trailing evac/DMA queue
                        if i < nkb - 6:
                            for _ in range(2 if i % 2 == 0 else 1):
                                if wo_q:
                                    wo_q.pop(0)()
                    elif emitted == 0 and wo_q:
                        wo_q.pop(0)()
                    pv(kb, pts.pop(kb), i == 0, i == nkb - 1)
                # drain any leftover projection work before the boundary
                while pieces:
                    pieces.pop(0)()
                if ptail is not None:
                    ptail()
                while vts:
                    vts.pop(0)()
                if last:
                    finish_a_last()
                    while wo_q:
                        wo_q.pop(0)(True)
                    finish_b_last()
                else:
                    prev = finish_a
                    wo_pending[ci] = wo_pieces()

    nc.finalize()
    return nc


def prep_inputs(x, Wq, Wk, Wv, Wo, token_positions, L=4096):
    """Host-side sharding + layout prep. Returns per-core input maps."""
    x = np.asarray(x, dtype=np.float32)
    Wq = np.asarray(Wq, dtype=np.float32)
    Wk = np.asarray(Wk, dtype=np.float32)
    Wv = np.asarray(Wv, dtype=np.float32)
    Wo = np.asarray(Wo, dtype=np.float32)
    pos = np.asarray(token_positions)[0].astype(np.float64)

    xt = np.ascontiguousarray(x[0].T).astype(np.float16)   # [D, L]
    i = np.arange(HEAD_DIM // 2, dtype=np.float64)
    freq = THETA ** (-2.0 * i / HEAD_DIM)                  # [32]
    ang = pos[:, None] * freq[None, :]                     # [L, 32]
    cos = np.cos(ang).T
    sin = np.sin(ang).T
    c64 = np.concatenate([cos, cos], axis=0)               # [64, L]
    s64 = np.concatenate([-sin, sin], axis=0)
    ctab = np.concatenate([c64, c64], axis=0)              # [128, L]
    s3tab = np.concatenate([s64, s64], axis=0)
    swapperm = np.concatenate([np.arange(32, 64), np.arange(0, 32),
                               np.arange(96, 128), np.arange(64, 96)])
    s3sw = s3tab[swapperm]   # row-swapped S3 for the fused early-chunk path
    cs2 = np.ascontiguousarray(
        np.stack([ctab, s3tab], axis=1)).astype(np.float16)   # [128, 2, L]

    perm = np.concatenate([np.arange(0, 64, 2), np.arange(1, 64, 2)])
    tri = (np.arange(128)[None, :] >= np.arange(128)[:, None]).astype(np.float16)
    identlo = np.zeros((128, 64), dtype=np.float16)
    identlo[np.arange(128), np.arange(128) % 64] = 1.0
    auxm = np.concatenate([tri, identlo, s3sw[:, 0:QC]], axis=1)
    auxm = np.ascontiguousarray(auxm).astype(np.float16)   # [128, 192+QC]

    in_maps = []
    for c in range(N_CORES):
        h0, h1, g = 2 * c, 2 * c + 1, c // 2
        qrows = np.concatenate([64 * h0 + perm, 64 * h1 + perm])
        # weight layouts pre-arranged as [p, dc, m] so the load DMA is one
        # contiguous 2KB-per-partition transfer
        wqt = np.ascontiguousarray(
            Wq[qrows, :].T.reshape(8, 128, 128).transpose(1, 0, 2)
        ).astype(np.float16)
        kv = np.concatenate([Wk[64 * g + perm, :], Wv[64 * g:64 * g + 64, :]], axis=0)
        wkvt = np.ascontiguousarray(
            kv.T.reshape(8, 128, 128).transpose(1, 0, 2)).astype(np.float16)
        attnrows = np.concatenate([np.arange(64 * h0, 64 * h0 + 64),
                                   np.arange(64 * h1, 64 * h1 + 64)])
        wo01 = np.ascontiguousarray(
            Wo[:, attnrows].T.reshape(128, 8, 128)).astype(np.float16)
        in_maps.append(dict(xt=xt, wqt=wqt, wkvt=wkvt, wo01=wo01,
                            cs2=cs2, aux=auxm))
    return in_maps


_NC_CACHE = {}


def _get_nc(L=4096):
    if L not in _NC_CACHE:
        _NC_CACHE[L] = build_kernel(L)
    return _NC_CACHE[L]


def kernel(x, Wq, Wk, Wv, Wo, token_positions):
    B, L, D = np.asarray(x).shape
    nc = _get_nc(L)
    in_maps = prep_inputs(x, Wq, Wk, Wv, Wo, token_positions, L=L)
    res = run_bass_kernel_spmd(nc, in_maps, list(range(N_CORES)))
    y = np.zeros((D_MODEL, L), dtype=np.float32)
    for r in res.results:
        y += r["yt"].astype(np.float32)
    return np.ascontiguousarray(y.T)[None].astype(np.float32)


# revision 40
# speedup vs baseline: 1.0094x; 1.0021x over previous
"""Trainium2 Bass kernel: GQA multi-head self-attention (B=1, L=4096, D=1024,
16 Q heads, 4 KV heads, head_dim 64, interleaved RoPE, causal softmax).

Sharding: 2 query heads + their (shared) KV head per core, 8 cores.
Each core computes a full-shape partial output Y_c.T = (attn_c @ Wo_c.T).T
(Megatron row-parallel style); the host sums the 8 partials.

Device-side design (per core):
  - x is fed pre-transposed (xT [D, L], fp16) so projection matmuls stream
    natural SBUF tiles; matmul operands are fp16 (1 cycle/row on the PE),
    accumulation stays fp32 in PSUM.
  - Q.T/K.T are produced in a "half-split" head-dim order (even dims then odd
    dims per head, via host-permuted weight rows) so RoPE's rotate-pair becomes
    a 32-partition block swap, done with SBUF->SBUF DMAs.
  - Attention runs in the S.T = K @ Q.T orientation: scores land in PSUM as
    [k=128, 2, q] tiles (both heads in one tile), exp runs on the scalar
    engine straight out of PSUM, and PV uses [V | ones] as the stationary
    operand so softmax denominators come out as row 64 of the PV accumulator
    for free. Diagonal key-blocks compute only the causally live query columns
    (matmul, exp and PV all narrowed).
  - Softmax normalization: DVE reciprocal straight off the PSUM denominator
    row, gpsimd partition_broadcast to replicate it across 64 partitions, one
    fused [128, q] attention-out tile so the output projection is 8 single
    (contraction-128) matmuls per chunk.
  - No max-subtraction pass: scores are O(1) here, exp cannot overflow, and
    softmax is shift-invariant so the result matches the reference.
  - Chunks have variable width: the first 512 columns are processed as two
    256-wide chunks so the startup pipeline (x DMA -> proj -> RoPE -> QK)
    fills in half the time; early-chunk PSUM evacuation runs on the (then
    idle) scalar engine to shorten the serial DVE RoPE chain.
  - Emission is software-pipelined: QK^T/exp run two key-blocks ahead of PV,
    and each chunk's normalize + output projection is deferred until the next
    chunk's first key-blocks are in flight, its 8 output-projection matmuls
    spread one-per-key-block so the PSUM ring never blocks the in-order PE
    stream. Non-final chunks store the projected output in one [128, 8, qw]
    staging tile and issue a single batched DMA; the final chunk uses paired
    PSUM tiles with alternating DVE/scalar evacuation and SP/gpsimd DMA
    queues to shorten the drain tail.
"""

import sys

for _p in ("/opt/trn_rl_repo",):
    if _p not in sys.path:
        sys.path.insert(0, _p)

import numpy as np

import concourse.bacc as bacc
import concourse.mybir as mybir
import concourse.tile as tile
from concourse.bass_utils import run_bass_kernel_spmd

F32 = mybir.dt.float32
F16 = mybir.dt.float16

D_MODEL = 1024
NUM_HEADS = 16
NUM_KV_HEADS = 4
HEAD_DIM = 64
THETA = 10000.0
N_CORES = 8
QC = 512          # max query chunk (free dim of S.T tiles per head)
KB = 128          # key block (partition dim of S.T tiles)


def chunk_bounds(L):
    """(q0, qw) per chunk: first 512 cols as two 256-wide chunks, then 512s."""
    out = [(0, 512)]
    q = 512
    while q < L:
        out.append((q, 512))
        q += 512
    return out


def build_kernel(L=4096):
    """One-core SPMD program. Handles its 2 query heads + 1 shared KV head."""
    nc = bacc.Bacc(None, target_bir_lowering=False)
    NT = L // KB          # number of 128-row key blocks / V tiles
    BOUNDS = chunk_bounds(L)
    NCH = len(BOUNDS)

    xt = nc.dram_tensor("xt", [D_MODEL, L], F16, kind="ExternalInput")
    wqt = nc.dram_tensor("wqt", [128, 8, 128], F16, kind="ExternalInput")
    wkvt = nc.dram_tensor("wkvt", [128, 8, 128], F16, kind="ExternalInput")
    wo01 = nc.dram_tensor("wo01", [128, 8, 128], F16, kind="ExternalInput")
    cs2 = nc.dram_tensor("cs2", [128, 2, L], F16, kind="ExternalInput")
    # aux packs [tri | identlo | cs0-swapped-S] so startup needs one small DMA
    aux = nc.dram_tensor("aux", [128, 192 + QC], F16, kind="ExternalInput")
    yt = nc.dram_tensor("yt", [D_MODEL, L], F16, kind="ExternalOutput")

    xt_r = xt.rearrange("(dc p) l -> p dc l", p=128)      # [128, 8, L]
    yt_r = yt.rearrange("(dc p) l -> p dc l", p=128)      # [128, 8, L]

    with tile.TileContext(nc) as tc:
        with (
            tc.tile_pool(name="consts", bufs=1) as consts,
            tc.tile_pool(name="big", bufs=1) as big,
            tc.tile_pool(name="xin", bufs=4) as xin,
            tc.tile_pool(name="work", bufs=5) as work,
            tc.tile_pool(name="ybp", bufs=2) as ybp,
            tc.tile_pool(name="ylast", bufs=8) as ylast,
            tc.tile_pool(name="ptp", bufs=14) as ptp,
            tc.tile_pool(name="stp", bufs=2, space="PSUM") as stp,
            tc.tile_pool(name="otp", bufs=2, space="PSUM") as otp,
            tc.tile_pool(name="mp", bufs=2, space="PSUM") as mp,
        ):
            # ---- constants in SBUF ----
            wqt_s = consts.tile([128, 8, 128], F16, tag="wqt")
            wkvt_s = consts.tile([128, 8, 128], F16, tag="wkvt")
            wo01_s = consts.tile([128, 8, 128], F16, tag="wo01")
            cs_s = consts.tile([128, 2, L], F16, tag="cs")
            aux_s = consts.tile([128, 192 + QC], F16, tag="aux")
            tri_s = aux_s[:, 0:128]
            identlo_s = aux_s[:, 128:192]
            cssw_s = aux_s[:, 192:192 + QC]

            def load_late_consts():
                nc.sync.dma_start(out=wo01_s, in_=wo01[:, :, :])

            # ---- persistent per-core activations ----
            qtrope = big.tile([128, L], F16, tag="qtrope")      # [2*64 halfsplit d, L]
            kt2 = big.tile([128, L], F16, tag="kt2")            # K.T duplicated twice
            vn = big.tile([128, NT * 65], F16, tag="vn")        # [V | 1] blocks
            nc.gpsimd.memset(vn[:, 64::65], 1.0)                # just the ones columns

            xtiles = {}

            def proj_dma(ci):
                q0, qw = BOUNDS[ci]
                ls = slice(q0, q0 + qw)
                xbig = xin.tile([128, 8, QC], F16, tag="xt")
                if ci == 0:
                    # startup ordering: DMAs staged in matmul-consumption order
                    # (q dc0-3, kv dc0-3, q dc4-7, kv dc4-7) at fine grain so
                    # the first projection matmul starts after ~0.3MB
                    nc.sync.dma_start(out=wqt_s[:, 0:4, :], in_=wqt[:, 0:4, :])
                    nc.sync.dma_start(out=xbig[:, 0:2, 0:qw], in_=xt_r[:, 0:2, ls])
                    nc.sync.dma_start(out=xbig[:, 2:4, 0:qw], in_=xt_r[:, 2:4, ls])
                    nc.sync.dma_start(out=wkvt_s[:, 0:4, :], in_=wkvt[:, 0:4, :])
                    nc.sync.dma_start(out=wqt_s[:, 4:8, :], in_=wqt[:, 4:8, :])
                    nc.sync.dma_start(out=xbig[:, 4:6, 0:qw], in_=xt_r[:, 4:6, ls])
                    nc.sync.dma_start(out=xbig[:, 6:8, 0:qw], in_=xt_r[:, 6:8, ls])
                    nc.sync.dma_start(out=wkvt_s[:, 4:8, :], in_=wkvt[:, 4:8, :])
                    nc.sync.dma_start(out=cs_s[:, 0, ls], in_=cs2[:, 0, ls])
                    nc.sync.dma_start(out=aux_s, in_=aux[:, :])
                elif ci == 1:
                    # halved + cos/sin table between the halves: chunk 1's
                    # first projection matmuls start ~1.6us earlier and chunk
                    # 1's rope table is in place before its evac completes
                    nc.sync.dma_start(out=xbig[:, 0:4, 0:qw], in_=xt_r[:, 0:4, ls])
                    nc.sync.dma_start(out=cs_s[:, :, ls], in_=cs2[:, :, ls])
                    nc.sync.dma_start(out=xbig[:, 4:8, 0:qw], in_=xt_r[:, 4:8, ls])
                else:
                    nc.sync.dma_start(out=xbig[:, :, 0:qw], in_=xt_r[:, :, ls])
                    nc.sync.dma_start(out=cs_s[:, :, ls], in_=cs2[:, :, ls])
                xtiles[ci] = xbig

            def make_proj_pieces(ci):
                q0, qw = BOUNDS[ci]
                ls = slice(q0, q0 + qw)
                xbig = xtiles.pop(ci)
                psum = {}

                def mm_piece(kind, dc):
                    def run():
                        if not psum:
                            psum["qt"] = mp.tile([128, QC], F32, tag="mp",
                                                  name="qt_ps")
                            psum["kv"] = mp.tile([128, QC], F32, tag="mp",
                                                  name="kvt_ps")
                        ps = psum["qt"] if kind == "q" else psum["kv"]
                        w = wqt_s if kind == "q" else wkvt_s
                        nc.tensor.matmul(ps[:, 0:qw], w[:, dc, :],
                                         xbig[:, dc, 0:qw],
                                         start=(dc == 0), stop=(dc == 7))
                    return run

                # half-interleaved so the low x half can be consumed while the
                # high half's DMA is still in flight (matters for chunk 0)
                pieces = ([mm_piece("q", dc) for dc in range(4)]
                          + [mm_piece("kv", dc) for dc in range(4)]
                          + [mm_piece("q", dc) for dc in range(4, 8)]
                          + [mm_piece("kv", dc) for dc in range(4, 8)])

                early = q0 + qw <= QC
                fill_phase = q0 + qw <= 3 * QC
                sbuf = {}

                def tail_dve():
                    # evacuate PSUM (fp32 -> fp16); early chunks route the
                    # Q-side evac to the (idle) scalar engine so the DVE RoPE
                    # chain starts sooner
                    qtraw = work.tile([128, QC], F16, tag="qtraw")
                    kvts = work.tile([128, QC], F16, tag="kvts")
                    sbuf["kvts"] = kvts
                    nc.vector.tensor_copy(kvts[:, 0:qw], psum["kv"][:, 0:qw])
                    if fill_phase:
                        nc.scalar.copy(qtraw[:, 0:qw], psum["qt"][:, 0:qw])
                    else:
                        nc.vector.tensor_copy(qtraw[:, 0:qw], psum["qt"][:, 0:qw])
                    rope(qtraw, kvts)

                def vt_piece(t):
                    # V natural layout via PE transpose: kvts[64:128] = V.T
                    def run():
                        kvts = sbuf["kvts"]
                        vt_ps = mp.tile([128, 64], F16, tag="mp",
                                        name="vt_ps")
                        nc.tensor.transpose(vt_ps,
                                            kvts[64:128, 128 * t:128 * t + 128],
                                            identlo_s[64:128, :])
                        blk = q0 // KB + t
                        if fill_phase:
                            # keep the vt->vn evac off the rope-busy DVE so the
                            # mp PSUM ring frees quickly for the next projection
                            nc.scalar.copy(vn[:, 65 * blk:65 * blk + 64], vt_ps)
                        else:
                            nc.vector.tensor_copy(vn[:, 65 * blk:65 * blk + 64],
                                                  vt_ps)
                    return run

                vts = [vt_piece(t) for t in range(qw // 128)]

                def rope(qtraw, kvts):
                    # RoPE: rot = raw*C + swapped*S3. Chunk 0 sits on the
                    # startup critical path: fold the rotate-pair swap into the
                    # S3 muls via the row-swapped table (cssw) with
                    # partition-shifted DVE outputs -- no staging DMA latency.
                    # Later chunks have pipeline slack: stage the swap through
                    # SBUF->SBUF DMAs on the idle gpsimd queue (cs channel 1 is
                    # the plain S3 table).
                    t1 = work.tile([128, QC], F16, tag="t1")
                    t2 = work.tile([128, QC], F16, tag="t2")
                    t3 = work.tile([64, QC], F16, tag="t1")
                    t4 = work.tile([64, QC], F16, tag="t2")
                    rope_body(qtraw, kvts, t1, t2, t3, t4)

                def rope_body(qtraw, kvts, t1, t2, t3, t4):
                    if early:
                        # K-side first: kvts comes off the DVE evac, qtraw off
                        # the scalar engine in parallel
                        nc.vector.tensor_mul(t3[:, 0:qw], kvts[0:64, 0:qw],
                                             cs_s[0:64, 0, ls])
                        nc.vector.tensor_mul(t4[0:32, 0:qw], kvts[32:64, 0:qw],
                                             cssw_s[32:64, ls])
                        nc.vector.tensor_mul(t4[32:64, 0:qw], kvts[0:32, 0:qw],
                                             cssw_s[0:32, ls])
                        nc.vector.tensor_add(kt2[0:64, ls], t3[:, 0:qw],
                                             t4[:, 0:qw])
                        nc.vector.tensor_add(kt2[64:128, ls], t3[:, 0:qw],
                                             t4[:, 0:qw])
                        nc.vector.tensor_mul(t1[:, 0:qw], qtraw[:, 0:qw],
                                             cs_s[:, 0, ls])
                        for (a, b) in ((0, 32), (32, 0), (64, 96), (96, 64)):
                            nc.vector.tensor_mul(t2[a:a + 32, 0:qw],
                                                 qtraw[b:b + 32, 0:qw],
                                                 cssw_s[b:b + 32, ls])
                        nc.vector.tensor_add(qtrope[:, ls], t1[:, 0:qw],
                                             t2[:, 0:qw])
                    else:
                        nc.vector.tensor_mul(t1[:, 0:qw], qtraw[:, 0:qw],
                                             cs_s[:, 0, ls])
                        nc.vector.tensor_mul(t3[:, 0:qw], kvts[0:64, 0:qw],
                                             cs_s[0:64, 0, ls])
                        qts = work.tile([128, QC], F16, tag="qts")
                        for (a, b) in ((0, 32), (32, 0), (64, 96), (96, 64)):
                            nc.gpsimd.dma_start(out=qts[a:a + 32, 0:qw],
                                                in_=qtraw[b:b + 32, 0:qw])
                        kts = work.tile([64, QC], F16, tag="kts")
                        nc.gpsimd.dma_start(out=kts[0:32, 0:qw],
                                            in_=kvts[32:64, 0:qw])
                        nc.gpsimd.dma_start(out=kts[32:64, 0:qw],
                                            in_=kvts[0:32, 0:qw])
                        nc.vector.tensor_mul(t2[:, 0:qw], qts[:, 0:qw],
                                             cs_s[:, 1, ls])
                        nc.vector.tensor_mul(t4[:, 0:qw], kts[:, 0:qw],
                                             cs_s[0:64, 1, ls])
                        nc.vector.tensor_add(qtrope[:, ls], t1[:, 0:qw],
                                             t2[:, 0:qw])
                        # both kt2 halves written (second add = the "dup")
                        nc.vector.tensor_add(kt2[0:64, ls], t3[:, 0:qw],
                                             t4[:, 0:qw])
                        nc.vector.tensor_add(kt2[64:128, ls], t3[:, 0:qw],
                                             t4[:, 0:qw])

                return pieces, tail_dve, vts

            def proj_compute(ci):
                pieces, tail_dve, vts = make_proj_pieces(ci)
                for p in pieces:
                    p()
                tail_dve()
                for v in vts:
                    v()

            def make_chunk(ci):
                q0, qw = BOUNDS[ci]
                qs = slice(q0, q0 + qw)
                d0 = q0 // KB                  # first diagonal key block
                nkb = (q0 + qw) // KB
                state = {}

                def qk(kb):
                    ks = slice(KB * kb, KB * kb + KB)
                    lo = KB * (kb - d0) if kb > d0 else 0
                    qsn = slice(q0 + lo, q0 + qw)
                    st = stp.tile([128, 2, QC], F32, tag="st")
                    nc.tensor.matmul(st[:, 0, lo:qw], kt2[0:64, ks],
                                     qtrope[0:64, qsn], start=True, stop=True)
                    nc.tensor.matmul(st[:, 1, lo:qw], kt2[64:128, ks],
                                     qtrope[64:128, qsn], start=True, stop=True)
                    pt = ptp.tile([128, 2, QC], F16, tag="pt")
                    nc.scalar.activation(pt[:, :, lo:qw], st[:, :, lo:qw],
                                         mybir.ActivationFunctionType.Exp,
                                         scale=0.125)
                    if kb >= d0:
                        nc.vector.tensor_mul(pt[:, 0, lo:lo + KB],
                                             pt[:, 0, lo:lo + KB], tri_s)
                        nc.vector.tensor_mul(pt[:, 1, lo:lo + KB],
                                             pt[:, 1, lo:lo + KB], tri_s)
                    return pt

                def pv(kb, pt, is_first, is_last):
                    if is_first:
                        state["ot0"] = otp.tile([65, QC], F32, tag="ot", name="ot0")
                        state["ot1"] = otp.tile([65, QC], F32, tag="ot", name="ot1")
                    lo = KB * (kb - d0) if kb >= d0 else 0
                    vblk = vn[:, 65 * kb:65 * kb + 65]
                    nc.tensor.matmul(state["ot0"][:, lo:qw], vblk, pt[:, 0, lo:qw],
                                     start=is_first, stop=is_last,
                                     skip_group_check=True)
                    nc.tensor.matmul(state["ot1"][:, lo:qw], vblk, pt[:, 1, lo:qw],
                                     start=is_first, stop=is_last,
                                     skip_group_check=True)

                def finish_a_last():
                    """finish_a for the final chunk, pipelined in halves: all
                    reciprocals first, then per-half broadcast -> normalize so
                    the output projection starts on the first half while the
                    second half's broadcast is still on gpsimd."""
                    rc2 = work.tile([1, 2 * QC], F16, tag="rc2")
                    h = qw // 2
                    with nc.allow_low_precision(reason="softmax denom recip fp16"):
                        nc.vector.reciprocal(rc2[:, 0:qw], state["ot0"][64:65, 0:qw])
                        nc.vector.reciprocal(rc2[:, QC:QC + qw],
                                             state["ot1"][64:65, 0:qw])
                    rbc = work.tile([64, 2 * QC], F16, tag="rbc")
                    otn = work.tile([128, QC], F16, tag="otn")
                    for (a, b) in ((0, h), (h, qw)):
                        nc.gpsimd.partition_broadcast(rbc[:, a:b], rc2[:, a:b])
                        nc.gpsimd.partition_broadcast(rbc[:, QC + a:QC + b],
                                                      rc2[:, QC + a:QC + b])
                        nc.vector.tensor_mul(otn[0:64, a:b],
                                             state["ot0"][0:64, a:b],
                                             rbc[:, a:b])
                        nc.vector.tensor_mul(otn[64:128, a:b],
                                             state["ot1"][0:64, a:b],
                                             rbc[:, QC + a:QC + b])
                    state["otn"] = otn

                def finish_a():
                    # softmax denominators: reciprocal straight off the PSUM
                    # ones-row, then replicate across 64 partitions on gpsimd
                    rc2 = work.tile([1, 2 * QC], F16, tag="rc2")
                    with nc.allow_low_precision(reason="softmax denom recip fp16"):
                        nc.vector.reciprocal(rc2[:, 0:qw], state["ot0"][64:65, 0:qw])
                        nc.vector.reciprocal(rc2[:, QC:QC + qw],
                                             state["ot1"][64:65, 0:qw])
                    rbc = work.tile([64, 2 * QC], F16, tag="rbc")
                    nc.gpsimd.partition_broadcast(rbc[:, 0:qw], rc2[:, 0:qw])
                    nc.gpsimd.partition_broadcast(rbc[:, QC:QC + qw],
                                                  rc2[:, QC:QC + qw])
                    # normalize here (not in finish_b) so these DVE ops sit
                    # ahead of the next chunk's mask ops in the DVE stream --
                    # the next chunk's pv(0) blocks on them via the ot ring
                    otn = work.tile([128, QC], F16, tag="otn")
                    nc.vector.tensor_mul(otn[0:64, 0:qw], state["ot0"][0:64, 0:qw],
                                         rbc[:, 0:qw])
                    nc.vector.tensor_mul(otn[64:128, 0:qw],
                                         state["ot1"][0:64, 0:qw],
                                         rbc[:, QC:QC + qw])
                    state["otn"] = otn

                def wo_pieces():
                    """Eight deferrable output-projection pieces (one matmul +
                    evac each; the last also flushes the staging DMA). Legal
                    any time after this chunk's finish_a; deferred into later
                    chunks' key-block loops as PE filler."""
                    staging = {}

                    def mk(dc):
                        def run(act_evac=False):
                            if "ysb" not in staging:
                                staging["ysb"] = ybp.tile([128, 8, QC], F16,
                                                          tag="ysb",
                                                          name="ysbbig")
                            ysbbig = staging["ysb"]
                            yps = mp.tile([128, QC], F32, tag="mp",
                                          name="yps")
                            nc.tensor.matmul(yps[:, 0:qw], wo01_s[:, dc, :],
                                             state["otn"][:, 0:qw],
                                             start=True, stop=True)
                            if act_evac and dc % 2:
                                # tail context: scalar engine is past its last
                                # exp, split the evac load off the DVE
                                nc.scalar.copy(ysbbig[:, dc, 0:qw],
                                               yps[:, 0:qw])
                            else:
                                nc.vector.tensor_copy(ysbbig[:, dc, 0:qw],
                                                      yps[:, 0:qw])
                            # flush in halves: two 0.5MB DMAs overlap the rest
                            # of the pipeline better than one 1MB DMA at dc7
                            if dc == 3:
                                nc.sync.dma_start(out=yt_r[:, 0:4, qs],
                                                  in_=ysbbig[:, 0:4, 0:qw])
                            elif dc == 7:
                                nc.sync.dma_start(out=yt_r[:, 4:8, qs],
                                                  in_=ysbbig[:, 4:8, 0:qw])
                        return run

                    return [mk(dc) for dc in range(8)]

                def finish_b_last():
                    # final chunk: single-dc PSUMs first and last, whole
                    # score-PSUM tiles (now dead) for the middle pairs; each
                    # projection matmul runs as two half-width pieces so the
                    # first halves start while otn's second half is still
                    # normalizing; alternate DVE/scalar evacuation and
                    # SP/gpsimd DMA queues to shorten the drain tail
                    otn = state["otn"]
                    hw_ = qw // 2
                    yp0 = mp.tile([128, QC], F32, tag="mp", name="yp0")
                    yp1 = mp.tile([128, QC], F32, tag="mp", name="yp1")
                    ypA = stp.tile([128, 2, QC], F32, tag="st")
                    ypB = stp.tile([128, 2, QC], F32, tag="st")
                    plan = [(yp0[:, 0:qw], 0), (yp1[:, 0:qw], 1),
                            (ypA[:, 0, :], 2), (ypA[:, 1, :], 3),
                            (ypB[:, 0, :], 4), (ypB[:, 1, :], 5)]
                    for dst, dc in plan:
                        nc.tensor.matmul(dst[:, 0:hw_], wo01_s[:, dc, :],
                                         otn[:, 0:hw_], start=True, stop=True)
                    yp6 = mp.tile([128, QC], F32, tag="mp", name="yp6")
                    yp7 = mp.tile([128, QC], F32, tag="mp", name="yp7")
                    plan2 = [(yp6[:, 0:qw], 6), (yp7[:, 0:qw], 7)]
                    for dst, dc in plan + plan2:
                        nc.tensor.matmul(dst[:, hw_:qw], wo01_s[:, dc, :],
                                         otn[:, hw_:qw], start=True, stop=True)
                    for dst, dc in plan2:
                        nc.tensor.matmul(dst[:, 0:hw_], wo01_s[:, dc, :],
                                         otn[:, 0:hw_], start=True, stop=True)
                    ysb0 = ylast.tile([128, QC], F16, tag="ysb2")
                    nc.vector.tensor_copy(ysb0, yp0)
                    nc.sync.dma_start(out=yt_r[:, 0, qs], in_=ysb0)
                    ysb1 = ylast.tile([128, QC], F16, tag="ysb2")
                    nc.scalar.copy(ysb1, yp1)
                    nc.gpsimd.dma_start(out=yt_r[:, 1, qs], in_=ysb1)
                    ysbA = ylast.tile([128, 2, QC], F16, tag="ysbp")
                    nc.vector.tensor_copy(ysbA, ypA)
                    nc.sync.dma_start(out=yt_r[:, 2:4, qs], in_=ysbA)
                    ysbB = ylast.tile([128, 2, QC], F16, tag="ysbp")
                    nc.scalar.copy(ysbB, ypB)
                    nc.gpsimd.dma_start(out=yt_r[:, 4:6, qs], in_=ysbB)
                    ysb6 = ylast.tile([128, QC], F16, tag="ysb2")
                    nc.vector.tensor_copy(ysb6, yp6)
                    nc.sync.dma_start(out=yt_r[:, 6, qs], in_=ysb6)
                    ysb7 = ylast.tile([128, QC], F16, tag="ysb2")
                    nc.scalar.copy(ysb7, yp7)
                    nc.gpsimd.dma_start(out=yt_r[:, 7, qs], in_=ysb7)

                return (nkb, qk, pv, finish_a, wo_pieces, finish_b_last,
                        finish_a_last)

            proj_dma(0)
            proj_compute(0)
            proj_dma(1)
            # chunk 1's projection emitted before chunk 0's attention: fills
            # the PE while chunk 0's RoPE chain runs on DVE/scalar
            proj_compute(1)
            proj_dma(2)
            load_late_consts()
            proj_dma(3)
            # --- global PE-filler scheduler state ---
            # wo(ci) is deferred exactly WO_DELAY chunks (bounded by the otn
            # ring depth) so the late, projection-free chunks still get one
            # ~213ns filler matmul per key block against the scalar engine's
            # ~1038ns/kb exp cadence (PE qk+pv alone is only ~852ns/kb).
            WO_DELAY = 4
            wo_q = []              # ready filler pieces (FIFO)
            wo_pending = {}        # ci -> its 8 wo pieces, not yet released
            prev = None
            for ci in range(NCH):
                q0, qw = BOUNDS[ci]
                d0 = q0 // KB
                (nkb, qk, pv, finish_a, wo_pieces, finish_b_last,
                 finish_a_last) = make_chunk(ci)
                diags0 = [kb for kb in range(d0, nkb) if kb != 0]
                second = diags0[0] if diags0 else 1
                last = ci == NCH - 1
                # during pipeline fill the next chunk's projection goes ahead
                # of this chunk's (rope-gated) first qk in the in-order PE
                # stream; in steady state its 16 matmuls become fillers
                if 1 <= ci <= 2 and ci < NCH - 1:
                    proj_compute(ci + 1)
                    pieces, ptail, vts = [], None, []
                elif 3 <= ci < NCH - 1:
                    pieces, ptail, vts = make_proj_pieces(ci + 1)
                else:
                    pieces, ptail, vts = [], None, []
                pts = {}
                pts[0] = qk(0)
                if nkb > 1:
                    pts[second] = qk(second)
                if prev is not None:
                    prev()
                if ci + 4 < NCH:
                    proj_dma(ci + 4)
                # release wo work whose deferral window ends at this chunk
                rel = ci - WO_DELAY
                if rel in wo_pending:
                    wo_q.extend(wo_pending.pop(rel))
                if last:
                    for cj in sorted(wo_pending):
                        wo_q.extend(wo_pending.pop(cj))
                # diagonal k-blocks early: their masks leave the boundary's
                # critical path; block 0 stays first (full-width start=True)
                diags = [kb for kb in range(d0, nkb) if kb != 0]
                rest = [kb for kb in range(1, d0)]
                order = [0] + diags + rest
                vt_at = None
                for i, kb in enumerate(order):
                    if i + 2 < nkb:
                        pts[order[i + 2]] = qk(order[i + 2])
                    # one filler piece per key block, emitted before pv so a
                    # dependency-stalled pv doesn't idle the in-order PE
                    emitted = 0
                    while pieces and len(pieces) > max(0, nkb - 4 - i):
                        pieces.pop(0)()
                        emitted += 1
                    if pieces and emitted == 0:
                        pieces.pop(0)()
                        emitted = 1
                    if not pieces and ptail is not None:
                        ptail()
                        ptail = None
                        # +4: the transposes read kvts via the DVE evac in
                        # ptail; give it ~2 key blocks of pipeline slack so
                        # the in-order PE doesn't stall on them
                        vt_at = i + 4
                    if vt_at is not None and i >= vt_at and vts:
                        vts.pop(0)()
                    if last:
                        # front-load ~1.5 wo pieces/slot (DVE evac budget),
                        # none in the last slots, so the drain window has no
                        # trailing evac/DMA queue
                        if i < nkb - 6:
                            for _ in range(2 if i % 2 == 0 else 1):
                                if wo_q:
                                    wo_q.pop(0)()
                    elif emitted == 0 and wo_q:
                        wo_q.pop(0)()
                    pv(kb, pts.pop(kb), i == 0, i == nkb - 1)
                # drain any leftover projection work before the boundary
                while pieces:
                    pieces.pop(0)()
                if ptail is not None:
                    ptail()
                while vts:
                    vts.pop(0)()
                if last:
                    finish_a_last()
                    while wo_q:
                        wo_q.pop(0)(True)
                    finish_b_last()
                else:
                    prev = finish_a
                    wo_pending[ci] = wo_pieces()

    nc.finalize()
    return nc


def prep_inputs(x, Wq, Wk, Wv, Wo, token_positions, L=4096):
    """Host-side sharding + layout prep. Returns per-core input maps."""
    x = np.asarray(x, dtype=np.float32)
    Wq = np.asarray(Wq, dtype=np.float32)
    Wk = np.asarray(Wk, dtype=np.float32)
    Wv = np.asarray(Wv, dtype=np.float32)
    Wo = np.asarray(Wo, dtype=np.float32)
    pos = np.asarray(token_positions)[0].astype(np.float64)

    xt = np.ascontiguousarray(x[0].T).astype(np.float16)   # [D, L]
    i = np.arange(HEAD_DIM // 2, dtype=np.float64)
    freq = THETA ** (-2.0 * i / HEAD_DIM)                  # [32]
    ang = pos[:, None] * freq[None, :]                     # [L, 32]
    cos = np.cos(ang).T
    sin = np.sin(ang).T
    c64 = np.concatenate([cos, cos], axis=0)               # [64, L]
    s64 = np.concatenate([-sin, sin], axis=0)
    ctab = np.concatenate([c64, c64], axis=0)              # [128, L]
    s3tab = np.concatenate([s64, s64], axis=0)
    swapperm = np.concatenate([np.arange(32, 64), np.arange(0, 32),
                               np.arange(96, 128), np.arange(64, 96)])
    s3sw = s3tab[swapperm]   # row-swapped S3 for the fused early-chunk path
    cs2 = np.ascontiguousarray(
        np.stack([ctab, s3tab], axis=1)).astype(np.float16)   # [128, 2, L]

    perm = np.concatenate([np.arange(0, 64, 2), np.arange(1, 64, 2)])
    tri = (np.arange(128)[None, :] >= np.arange(128)[:, None]).astype(np.float16)
    identlo = np.zeros((128, 64), dtype=np.float16)
    identlo[np.arange(128), np.arange(128) % 64] = 1.0
    auxm = np.concatenate([tri, identlo, s3sw[:, 0:QC]], axis=1)
    auxm = np.ascontiguousarray(auxm).astype(np.float16)   # [128, 192+QC]

    in_maps = []
    for c in range(N_CORES):
        h0, h1, g = 2 * c, 2 * c + 1, c // 2
        qrows = np.concatenate([64 * h0 + perm, 64 * h1 + perm])
        # weight layouts pre-arranged as [p, dc, m] so the load DMA is one
        # contiguous 2KB-per-partition transfer
        wqt = np.ascontiguousarray(
            Wq[qrows, :].T.reshape(8, 128, 128).transpose(1, 0, 2)
        ).astype(np.float16)
        kv = np.concatenate([Wk[64 * g + perm, :], Wv[64 * g:64 * g + 64, :]], axis=0)
        wkvt = np.ascontiguousarray(
            kv.T.reshape(8, 128, 128).transpose(1, 0, 2)).astype(np.float16)
        attnrows = np.concatenate([np.arange(64 * h0, 64 * h0 + 64),
                                   np.arange(64 * h1, 64 * h1 + 64)])
        wo01 = np.ascontiguousarray(
            Wo[:, attnrows].T.reshape(128, 8, 128)).astype(np.float16)
        in_maps.append(dict(xt=xt, wqt=wqt, wkvt=wkvt, wo01=wo01,
                            cs2=cs2, aux=auxm))
    return in_maps


_NC_CACHE = {}


def _get_nc(L=4096):
    if L not in _NC_CACHE:
        _NC_CACHE[L] = build_kernel(L)
    return _NC_CACHE[L]


def kernel(x, Wq, Wk, Wv, Wo, token_positions):
    B, L, D = np.asarray(x).shape
    nc = _get_nc(L)
    in_maps = prep_inputs(x, Wq, Wk, Wv, Wo, token_positions, L=L)
    res = run_bass_kernel_spmd(nc, in_maps, list(range(N_CORES)))
    y = np.zeros((D_MODEL, L), dtype=np.float32)
    for r in res.results:
        y += r["yt"].astype(np.float32)
    return np.ascontiguousarray(y.T)[None].astype(np.float32)
